# revision 9
# baseline (speedup 1.0000x reference)
"""MoE layer (24 experts, top-2 routing) on 8 Trainium2 NeuronCores.

Expert-parallel sharding: the host computes the gate routing (scores -> top-2
-> softmax combine weights), then dispatches each expert's tokens to the core
that owns the expert (3 experts per core, count-balanced by a sort-descending
assignment).  Each core runs one SPMD Bass/Tile program that, for each of its
3 expert slots, computes

    H^T[f, t] = gelu(w1^T-contract(x^T) + b1)      (MM1, K = d_model = 1024)
    Y^T[d, t] = w2^T-contract(H^T) + b2            (MM2, K = d_ff    = 4096)
    out       = Y^T * gate_weight[t]

with tokens on the matmul FREE dim, so per-expert token counts need no
128-padding (capacity = max count per slot across cores, rounded to even).
The host scatter-adds the per-expert outputs back into the [T, d] output
(the "combine" side of the all-to-all).

Matmuls run in bf16 (weights, x, and h), accumulating in fp32 PSUM: the PE
streams 1 row/cycle either way, but bf16 halves the dominant HBM traffic -
the expert weights (100 MB/core fp32 -> 50 MB/core bf16), turning the kernel
from DMA-bound into PE-bound.  Matmul rel-error ~3e-3, far inside the 2e-2
gate (fp8 was measured at 5e-2 end-to-end - over the gate - so bf16 is the
fastest admissible dtype).  Weight DMAs are batched into 2 MiB transfers
(8 MM1 f-tiles / 2 MM2 d-tiles per DMA), alternating between the two HWDGE
rings (SP and ACT issuing engines); output stores go through SWDGE (gpsimd)
so their compute-gated semaphore waits never block the load rings.  The MM2
epilogue is a single fused DVE op reading PSUM: yo = (py + b2) * gate.
Biases, gate weights, PSUM, and the output stay fp32.

Per core and per pass this streams 512*sum(caps) ~ 549k PE columns
(~229 us at 2.4 GHz) + 1536 weight-block swaps; measured ~235 us/pass
burst, ~283 us sustained (power-throttled).  The expert->(core,slot)
assignment (sort by count desc, slot j = ranks [8j,8j+8)) minimizes
sum-of-slot-capacities: caps >= (c0, c8, c16) elementwise by pigeonhole,
and sorted grouping achieves that bound.

Host-side work is routing/dispatch/combine only (index math, gather,
scatter-add); all FLOPs of the MoE layer itself (both matmuls, gelu, biases,
gate weighting) run on device.
"""

import sys

for _p in ("/opt/trn_rl_repo", "/root/.axon_site/_ro/trn_rl_repo"):
    if _p not in sys.path:
        sys.path.append(_p)

import ml_dtypes
import numpy as np

import concourse.tile as tile
from concourse import bacc, mybir
from concourse.bass_utils import run_bass_kernel_spmd

B, S, D, FF, E, TOPK = 4, 1024, 1024, 4096, 24, 2
T = B * S
P = 128
KT1 = D // P     # 8  k-subtiles for MM1
MT1 = FF // P    # 32 f-tiles (MM1 output partition tiles)
KT2 = FF // P    # 32 k-subtiles for MM2
MT2 = D // P     # 8  d-tiles (MM2 output partition tiles)
W1G = 8          # MM1 f-tiles per weight DMA (2 MiB bf16 per transfer)
G1 = MT1 // W1G  # 4 w1 DMA groups
W2G = 2          # MM2 d-tiles per weight DMA (2 MiB bf16 per transfer)
G2 = MT2 // W2G  # 4 w2 DMA groups
N_CORES = 8
SLOTS = E // N_CORES  # 3 experts per core

BF16 = mybir.dt.bfloat16
F8E3 = mybir.dt.float8e3
F32 = mybir.dt.float32
NP_BF16 = ml_dtypes.bfloat16
NP_F8E3 = ml_dtypes.float8_e3m4

# Weights are stored and matmul'd as fp8 E3M4 (4 mantissa bits) against bf16
# moving operands - the PE allows mixed dtypes and runs at bf16 speed (1
# col/cycle), so this halves the dominant weight HBM traffic (50->25 MB/core
# per pass) and cuts PE multiplier toggling, while the global dequant scales
# fold into existing epilogues (ACT `scale` for MM1, gate-weight multiply for
# MM2) at zero device cost.  Measured end-to-end rel-err ~1.75e-2 (gate 2e-2);
# weights uniform +-1/sqrt(D) quantize to +-15.5 with a single global scale.
E3_MAX = 15.5

_program_cache: dict = {}


def _build_program(caps, s1=1.0, loop_reps=None, bench_internal=False, hw_loop=0):
    """One SPMD program: SLOTS expert slots with token capacities caps[j].

    loop_reps: replicate the body N times (benchmark-only, to measure the
    steady-state device time via a wall-clock slope over N).
    bench_internal: benchmark-only - every tensor lives in internal DRAM
    scratch (plus one tiny ExternalOutput so the program has I/O), so
    wall-clock timing excludes host<->device shipping while keeping
    an identical per-rep instruction stream and DMA traffic.
    hw_loop: benchmark-only - wrap the loop_reps unrolled reps in a hardware
    For_i loop of this many iterations (total reps = loop_reps * hw_loop).
    The relay's ~100 ms per-call overhead partially overlaps device
    execution, so honest wall-clock slopes need hundreds of ms of device
    time per call; the HW loop's back-edge barrier adds a small real cost
    per iteration (measured time is, if anything, pessimistic).
    """
    nc = bacc.Bacc("TRN2", target_bir_lowering=False, debug=False)

    kin = "Internal" if bench_internal else "ExternalInput"
    kout = "Internal" if bench_internal else "ExternalOutput"
    sfx = "_int" if bench_internal else ""
    w1t = nc.dram_tensor("w1t" + sfx, (SLOTS, G1, P, W1G, KT1, P), F8E3, kind=kin)
    w2t = nc.dram_tensor("w2t" + sfx, (SLOTS, G2, P, W2G, KT2, P), F8E3, kind=kin)
    b1t = nc.dram_tensor("b1t" + sfx, (SLOTS, P, MT1), F32, kind=kin)
    b2t = nc.dram_tensor("b2t" + sfx, (SLOTS, P, MT2), F32, kind=kin)
    xgs = [nc.dram_tensor(f"xg{j}" + sfx, (P, KT1, caps[j]), BF16, kind=kin)
           for j in range(SLOTS)]
    gws = [nc.dram_tensor(f"gw{j}" + sfx, (P, caps[j]), F32, kind=kin)
           for j in range(SLOTS)]
    ygs = [nc.dram_tensor(f"yg{j}" + sfx, (MT2, P, caps[j]), F32, kind=kout)
           for j in range(SLOTS)]
    tick = (nc.dram_tensor("tick", (1, 2), F32, kind="ExternalOutput")
            if bench_internal else None)

    import contextlib

    with tile.TileContext(nc) as tc:
        with tc.tile_pool(name="xg", bufs=SLOTS + 1) as xg_pool, \
             tc.tile_pool(name="gw", bufs=SLOTS + 1) as gw_pool, \
             tc.tile_pool(name="bias", bufs=SLOTS + 1) as bias_pool, \
             tc.tile_pool(name="w1", bufs=3) as w1_pool, \
             tc.tile_pool(name="w2", bufs=3) as w2_pool, \
             tc.tile_pool(name="h", bufs=MT1) as h_pool, \
             tc.tile_pool(name="epi", bufs=4) as epi_pool, \
             tc.tile_pool(name="psa", bufs=4, space="PSUM") as psa, \
             tc.tile_pool(name="psb", bufs=4, space="PSUM") as psb:
            if tick is not None:
                tk = gw_pool.tile([P, 2], F32, tag="tick")
                nc.vector.memset(tk[:], 0.0)
            loop_cm = (tc.For_i(0, hw_loop, 1,
                               hint_engines=(mybir.EngineType.PE,))
                       if hw_loop else contextlib.nullcontext())
            with loop_cm:
                for _rep in range(loop_reps or 1):
                        dma_rr = [0]

                        def wdma(dst, src):
                            # alternate weight DMAs across the two HWDGE rings
                            eng = nc.scalar if (dma_rr[0] % 2) else nc.sync
                            dma_rr[0] += 1
                            eng.dma_start(dst, src)

                        # Slot preloads: activations/gates ride the SCALAR
                        # ring so the first w1 DMA leads the SYNC ring and the
                        # PE's first matmul waits only max(xg0, w1g0), not
                        # their sum.  Slots 1-2 preload lazily during the
                        # previous expert's compute (plenty of slack).
                        slot_in = []

                        def preload(j):
                            C = caps[j]
                            xg_sb = xg_pool.tile([P, KT1, C], BF16, tag="xg")
                            nc.scalar.dma_start(xg_sb[:], xgs[j].ap()[:])
                            gw_sb = gw_pool.tile([P, C], F32, tag="gw")
                            nc.scalar.dma_start(gw_sb[:], gws[j].ap()[:])
                            b1_sb = bias_pool.tile([P, MT1], F32, tag="b1")
                            nc.sync.dma_start(b1_sb[:], b1t.ap()[j])
                            b2_sb = bias_pool.tile([P, MT2], F32, tag="b2")
                            nc.sync.dma_start(b2_sb[:], b2t.ap()[j])
                            slot_in.append((xg_sb, gw_sb, b1_sb, b2_sb))

                        preload(0)
                        for j in range(SLOTS):
                            C = caps[j]
                            xg_sb, gw_sb, b1_sb, b2_sb = slot_in[j]

                            # Phase A: H^T tiles, one 128-row f-tile at a time.
                            h_tiles = []
                            for g in range(G1):
                                w1_sb = w1_pool.tile([P, W1G, KT1, P], F8E3, tag="w1")
                                wdma(w1_sb[:], w1t.ap()[j, g])
                                for mi in range(W1G):
                                    m = g * W1G + mi
                                    ph = psa.tile([P, C], F32, tag="psa")
                                    for k in range(KT1):
                                        nc.tensor.matmul(ph[:], w1_sb[:, mi, k, :],
                                                         xg_sb[:, k, :],
                                                         start=(k == 0),
                                                         stop=(k == KT1 - 1))
                                    h_sb = h_pool.tile([P, C], BF16, tag="h")
                                    nc.scalar.activation(h_sb[:], ph[:],
                                                         mybir.ActivationFunctionType.Gelu,
                                                         bias=b1_sb[:, m:m + 1],
                                                         scale=float(s1))
                                    h_tiles.append(h_sb)

                            # Phase B: Y^T tiles; epilogue adds b2, scales by gate.
                            for go in range(G2):
                                w2_sb = w2_pool.tile([P, W2G, KT2, P], F8E3, tag="w2")
                                wdma(w2_sb[:], w2t.ap()[j, go])
                                if go == 0 and j + 1 < SLOTS:
                                    preload(j + 1)
                                for mi in range(W2G):
                                    mo = go * W2G + mi
                                    py = psb.tile([P, C], F32, tag="psb")
                                    for k in range(KT2):
                                        nc.tensor.matmul(py[:], w2_sb[:, mi, k, :],
                                                         h_tiles[k][:],
                                                         start=(k == 0),
                                                         stop=(k == KT2 - 1))
                                    # fused epilogue on DVE: yo = (py + b2) * gw
                                    # (single PSUM reader in phase B; ACT keeps gelu)
                                    yo = epi_pool.tile([P, C], F32, tag="yo")
                                    nc.vector.scalar_tensor_tensor(
                                        yo[:], py[:], b2_sb[:, mo:mo + 1], gw_sb[:],
                                        op0=mybir.AluOpType.add,
                                        op1=mybir.AluOpType.mult)
                                    # store via SWDGE (gpsimd) so the compute-gated
                                    # store's sem-wait never blocks the HWDGE load
                                    # rings at rep/expert boundaries
                                    nc.gpsimd.dma_start(ygs[j].ap()[mo], yo[:])
            if tick is not None:
                # tiny I/O so the bench program has an ExternalOutput
                nc.sync.dma_start(tick.ap()[:], tk[0:1, 0:2])
    nc.compile()
    return nc


def _route(x2d, gate_w, gate_b):
    """fp32 gate scores -> top-2 indices -> softmax combine weights."""
    scores = x2d @ gate_w + gate_b                               # [T, E]
    topi = np.argsort(-scores, axis=1, kind="stable")[:, :TOPK]  # [T, 2]
    topv = np.take_along_axis(scores, topi, axis=1)
    g = np.exp(topv - topv.max(axis=1, keepdims=True))
    g = g / g.sum(axis=1, keepdims=True)
    return topi, g.astype(np.float32)


def kernel(x, gate_w, gate_b, w1, b1, w2, b2):
    x = np.ascontiguousarray(np.asarray(x, dtype=np.float32))
    gate_w = np.asarray(gate_w, dtype=np.float32)
    gate_b = np.asarray(gate_b, dtype=np.float32)
    w1 = np.asarray(w1, dtype=np.float32)
    b1 = np.asarray(b1, dtype=np.float32)
    w2 = np.asarray(w2, dtype=np.float32)
    b2 = np.asarray(b2, dtype=np.float32)

    x2d = x.reshape(T, D)
    topi, gates = _route(x2d, gate_w, gate_b)

    # Token list and combine weight per expert (token order preserved).
    idx_e = [np.nonzero(topi == e)[0] for e in range(E)]
    gv_e = []
    for e in range(E):
        rows = topi == e                       # [T, 2] bool, <=1 True per row
        sel = rows.any(axis=1)
        gv_e.append(gates[sel, :][rows[sel, :]].astype(np.float32))
    counts = np.array([len(i) for i in idx_e])

    # Balance experts over (core, slot): sort by count descending; slot j
    # holds ranks [8j, 8j+8).  Slot capacity = max count in the slot,
    # rounded up to even.
    order = np.argsort(-counts, kind="stable")
    slot_expert = np.empty((N_CORES, SLOTS), dtype=int)
    caps = []
    for j in range(SLOTS):
        ranks = order[j * N_CORES:(j + 1) * N_CORES]
        slot_expert[:, j] = ranks
        cmax = int(counts[ranks].max())
        caps.append(cmax + (cmax & 1))
    caps = tuple(caps)

    # global e3m4 weight scales (identical across experts: iid uniform)
    s1 = float(np.abs(w1).max() / E3_MAX)
    s2 = float(np.abs(w2).max() / E3_MAX)

    key = (caps, np.float32(s1).item())
    if key not in _program_cache:
        _program_cache[key] = _build_program(caps, s1=s1)
    nc = _program_cache[key]

    xTb = np.ascontiguousarray(x2d.T).astype(NP_BF16)      # [D, T] bf16
    # e3m4-quantized weights (values in +-15.5, dequant scales fold into the
    # ACT scale (mm1) and the host-side gw/b2 rescale (mm2))
    w1q = np.clip(w1 / s1, -E3_MAX, E3_MAX).astype(NP_F8E3)
    w2q = np.clip(w2 / s2, -E3_MAX, E3_MAX).astype(NP_F8E3)
    in_maps = []
    for c in range(N_CORES):
        m = {}
        w1c = np.empty((SLOTS, G1, P, W1G, KT1, P), NP_F8E3)
        w2c = np.empty((SLOTS, G2, P, W2G, KT2, P), NP_F8E3)
        b1c = np.empty((SLOTS, P, MT1), np.float32)
        b2c = np.empty((SLOTS, P, MT2), np.float32)
        for j in range(SLOTS):
            e = int(slot_expert[c, j])
            C = caps[j]
            n = int(counts[e])
            xg = np.zeros((P, KT1, C), NP_BF16)
            xg[:, :, :n] = xTb[:, idx_e[e]].reshape(KT1, P, n).transpose(1, 0, 2)
            m[f"xg{j}"] = xg
            gw = np.zeros((C,), np.float32)
            gw[:n] = gv_e[e] * s2
            m[f"gw{j}"] = np.broadcast_to(gw, (P, C)).copy()
            # weight tiles in the exact SBUF layouts for single clean DMAs
            t1 = w1q[e].reshape(KT1, P, MT1, P).transpose(2, 1, 0, 3)
            w1c[j] = t1.reshape(G1, W1G, P, KT1, P).transpose(0, 2, 1, 3, 4)
            t2 = w2q[e].reshape(KT2, P, MT2, P).transpose(2, 1, 0, 3)
            w2c[j] = t2.reshape(G2, W2G, P, KT2, P).transpose(0, 2, 1, 3, 4)
            b1c[j] = b1[e].reshape(MT1, P).T
            b2c[j] = (b2[e] / s2).reshape(MT2, P).T
        m["w1t"] = w1c
        m["w2t"] = w2c
        m["b1t"] = b1c
        m["b2t"] = b2c
        in_maps.append(m)

    res = run_bass_kernel_spmd(nc, in_maps, core_ids=list(range(N_CORES)))

    # Combine: scatter-add each expert's weighted outputs back to tokens.
    out = np.zeros((T, D), np.float32)
    for c in range(N_CORES):
        for j in range(SLOTS):
            e = int(slot_expert[c, j])
            n = int(counts[e])
            yg = res.results[c][f"yg{j}"].reshape(D, caps[j])
            out[idx_e[e], :] += yg[:, :n].T
    return out.reshape(B, S, D)



# revision 12
# speedup vs baseline: 1.1043x; 1.1043x over previous
"""MoE layer (24 experts, top-2 routing) on 8 Trainium2 NeuronCores.

Expert-parallel sharding: the host computes the gate routing (scores -> top-2
-> softmax combine weights), then dispatches each expert's tokens to the core
that owns the expert (3 experts per core, count-balanced by a sort-descending
assignment).  Each core runs one SPMD Bass/Tile program that, for each of its
3 expert slots, computes

    H^T[f, t] = gelu(w1^T-contract(x^T) + b1)      (MM1, K = d_model = 1024)
    Y^T[d, t] = w2^T-contract(H^T) + b2            (MM2, K = d_ff    = 4096)
    out       = Y^T * gate_weight[t]

with tokens on the matmul FREE dim, so per-expert token counts need no
128-padding (capacity = max count per slot across cores, rounded to even).
The host scatter-adds the per-expert outputs back into the [T, d] output
(the "combine" side of the all-to-all).

Matmuls run in bf16 (weights, x, and h), accumulating in fp32 PSUM: the PE
streams 1 row/cycle either way, but bf16 halves the dominant HBM traffic -
the expert weights (100 MB/core fp32 -> 50 MB/core bf16), turning the kernel
from DMA-bound into PE-bound.  Matmul rel-error ~3e-3, far inside the 2e-2
gate (fp8 was measured at 5e-2 end-to-end - over the gate - so bf16 is the
fastest admissible dtype).  Weight DMAs are batched into 2 MiB transfers
(8 MM1 f-tiles / 2 MM2 d-tiles per DMA), alternating between the two HWDGE
rings (SP and ACT issuing engines); output stores go through SWDGE (gpsimd)
so their compute-gated semaphore waits never block the load rings.  The MM2
epilogue is a single fused DVE op reading PSUM: yo = (py + b2) * gate.
Biases, gate weights, PSUM, and the output stay fp32.

Per core and per pass this streams 512*sum(caps) ~ 549k PE columns
(~229 us at 2.4 GHz) + 1536 weight-block swaps; measured ~235 us/pass
burst, ~283 us sustained (power-throttled).  The expert->(core,slot)
assignment (sort by count desc, slot j = ranks [8j,8j+8)) minimizes
sum-of-slot-capacities: caps >= (c0, c8, c16) elementwise by pigeonhole,
and sorted grouping achieves that bound.

Host-side work is routing/dispatch/combine only (index math, gather,
scatter-add); all FLOPs of the MoE layer itself (both matmuls, gelu, biases,
gate weighting) run on device.
"""

import os
import sys

for _p in ("/opt/trn_rl_repo", "/root/.axon_site/_ro/trn_rl_repo"):
    if _p not in sys.path:
        sys.path.append(_p)

import ml_dtypes
import numpy as np

import concourse.tile as tile
from concourse import bacc, mybir
from concourse.bass_utils import run_bass_kernel_spmd

B, S, D, FF, E, TOPK = 4, 1024, 1024, 4096, 24, 2
T = B * S
P = 128
KT1 = D // P     # 8  k-subtiles for MM1
MT1 = FF // P    # 32 f-tiles (MM1 output partition tiles)
KT2 = FF // P    # 32 k-subtiles for MM2
MT2 = D // P     # 8  d-tiles (MM2 output partition tiles)
W1G = 8          # MM1 f-tiles per weight DMA (2 MiB bf16 per transfer)
G1 = MT1 // W1G  # 4 w1 DMA groups
W2G = 2          # MM2 d-tiles per weight DMA (2 MiB bf16 per transfer)
G2 = MT2 // W2G  # 4 w2 DMA groups
N_CORES = 8
SLOTS = E // N_CORES  # 3 experts per core

BF16 = mybir.dt.bfloat16
F8E3 = mybir.dt.float8e3
F32 = mybir.dt.float32
NP_BF16 = ml_dtypes.bfloat16
NP_F8E3 = ml_dtypes.float8_e3m4

# Weights are stored and matmul'd as fp8 E3M4 (4 mantissa bits) against bf16
# moving operands - the PE allows mixed dtypes and runs at bf16 speed (1
# col/cycle), so this halves the dominant weight HBM traffic (50->25 MB/core
# per pass) and cuts PE multiplier toggling, while the global dequant scales
# fold into existing epilogues (ACT `scale` for MM1, gate-weight multiply for
# MM2) at zero device cost.  Measured end-to-end rel-err ~1.75e-2 (gate 2e-2);
# weights uniform +-1/sqrt(D) quantize to +-15.5 with a single global scale.
E3_MAX = 15.5

_program_cache: dict = {}


def _build_program(caps, s1=1.0, loop_reps=None, bench_internal=False, hw_loop=0):
    """One SPMD program: SLOTS expert slots with token capacities caps[j].

    loop_reps: replicate the body N times (benchmark-only, to measure the
    steady-state device time via a wall-clock slope over N).
    bench_internal: benchmark-only - every tensor lives in internal DRAM
    scratch (plus one tiny ExternalOutput so the program has I/O), so
    wall-clock timing excludes host<->device shipping while keeping
    an identical per-rep instruction stream and DMA traffic.
    hw_loop: benchmark-only - wrap the loop_reps unrolled reps in a hardware
    For_i loop of this many iterations (total reps = loop_reps * hw_loop).
    The relay's ~100 ms per-call overhead partially overlaps device
    execution, so honest wall-clock slopes need hundreds of ms of device
    time per call; the HW loop's back-edge barrier adds a small real cost
    per iteration (measured time is, if anything, pessimistic).
    """
    nc = bacc.Bacc("TRN2", target_bir_lowering=False, debug=False)

    kin = "Internal" if bench_internal else "ExternalInput"
    kout = "Internal" if bench_internal else "ExternalOutput"
    sfx = "_int" if bench_internal else ""
    w1t = nc.dram_tensor("w1t" + sfx, (SLOTS, G1, P, W1G, KT1, P), F8E3, kind=kin)
    w2t = nc.dram_tensor("w2t" + sfx, (SLOTS, G2, P, W2G, KT2, P), F8E3, kind=kin)
    b1t = nc.dram_tensor("b1t" + sfx, (SLOTS, P, MT1), F32, kind=kin)
    b2t = nc.dram_tensor("b2t" + sfx, (SLOTS, P, MT2), F32, kind=kin)
    xgs = [nc.dram_tensor(f"xg{j}" + sfx, (P, KT1, caps[j]), BF16, kind=kin)
           for j in range(SLOTS)]
    gws = [nc.dram_tensor(f"gw{j}" + sfx, (P, caps[j]), F32, kind=kin)
           for j in range(SLOTS)]
    ygs = [nc.dram_tensor(f"yg{j}" + sfx, (MT2, P, caps[j]), F32, kind=kout)
           for j in range(SLOTS)]
    tick = (nc.dram_tensor("tick", (1, 2), F32, kind="ExternalOutput")
            if bench_internal else None)

    import contextlib

    with tile.TileContext(nc) as tc:
        with tc.tile_pool(name="xg", bufs=SLOTS + 1) as xg_pool, \
             tc.tile_pool(name="gw", bufs=SLOTS + 1) as gw_pool, \
             tc.tile_pool(name="bias", bufs=SLOTS + 1) as bias_pool, \
             tc.tile_pool(name="w1", bufs=3) as w1_pool, \
             tc.tile_pool(name="w2", bufs=3) as w2_pool, \
             tc.tile_pool(name="h", bufs=MT1) as h_pool, \
             tc.tile_pool(name="epi", bufs=4) as epi_pool, \
             tc.tile_pool(name="psa", bufs=4, space="PSUM") as psa, \
             tc.tile_pool(name="psb", bufs=4, space="PSUM") as psb:
            if tick is not None:
                tk = gw_pool.tile([P, 2], F32, tag="tick")
                nc.vector.memset(tk[:], 0.0)
            loop_cm = (tc.For_i(0, hw_loop, 1,
                               hint_engines=(mybir.EngineType.PE,))
                       if hw_loop else contextlib.nullcontext())
            with loop_cm:
                for _rep in range(loop_reps or 1):
                        dma_rr = [0]

                        def wdma(dst, src):
                            # alternate weight DMAs across the two HWDGE rings
                            eng = nc.scalar if (dma_rr[0] % 2) else nc.sync
                            dma_rr[0] += 1
                            eng.dma_start(dst, src)

                        # Slot preloads: activations/gates ride the SCALAR
                        # ring so the first w1 DMA leads the SYNC ring and the
                        # PE's first matmul waits only max(xg0, w1g0), not
                        # their sum.  Slots 1-2 preload lazily during the
                        # previous expert's compute (plenty of slack).
                        slot_in = []

                        def preload(j):
                            C = caps[j]
                            xg_sb = xg_pool.tile([P, KT1, C], BF16, tag="xg")
                            nc.scalar.dma_start(xg_sb[:], xgs[j].ap()[:])
                            gw_sb = gw_pool.tile([P, C], F32, tag="gw")
                            nc.scalar.dma_start(gw_sb[:], gws[j].ap()[:])
                            b1_sb = bias_pool.tile([P, MT1], F32, tag="b1")
                            nc.sync.dma_start(b1_sb[:], b1t.ap()[j])
                            b2_sb = bias_pool.tile([P, MT2], F32, tag="b2")
                            nc.sync.dma_start(b2_sb[:], b2t.ap()[j])
                            slot_in.append((xg_sb, gw_sb, b1_sb, b2_sb))

                        preload(0)
                        for j in range(SLOTS):
                            C = caps[j]
                            xg_sb, gw_sb, b1_sb, b2_sb = slot_in[j]

                            # Phase A: H^T tiles, one 128-row f-tile at a time.
                            h_tiles = []
                            for g in range(G1):
                                w1_sb = w1_pool.tile([P, W1G, KT1, P], F8E3, tag="w1")
                                wdma(w1_sb[:], w1t.ap()[j, g])
                                for mi in range(W1G):
                                    m = g * W1G + mi
                                    ph = psa.tile([P, C], F32, tag="psa")
                                    for k in range(KT1):
                                        nc.tensor.matmul(ph[:], w1_sb[:, mi, k, :],
                                                         xg_sb[:, k, :],
                                                         start=(k == 0),
                                                         stop=(k == KT1 - 1))
                                    h_sb = h_pool.tile([P, C], BF16, tag="h")
                                    nc.scalar.activation(h_sb[:], ph[:],
                                                         mybir.ActivationFunctionType.Gelu,
                                                         bias=b1_sb[:, m:m + 1],
                                                         scale=float(s1))
                                    h_tiles.append(h_sb)

                            # Phase B: Y^T tiles; epilogue adds b2, scales by gate.
                            for go in range(G2):
                                w2_sb = w2_pool.tile([P, W2G, KT2, P], F8E3, tag="w2")
                                wdma(w2_sb[:], w2t.ap()[j, go])
                                if go == 0 and j + 1 < SLOTS:
                                    preload(j + 1)
                                for mi in range(W2G):
                                    mo = go * W2G + mi
                                    py = psb.tile([P, C], F32, tag="psb")
                                    for k in range(KT2):
                                        nc.tensor.matmul(py[:], w2_sb[:, mi, k, :],
                                                         h_tiles[k][:],
                                                         start=(k == 0),
                                                         stop=(k == KT2 - 1))
                                    # fused epilogue on DVE: yo = (py + b2) * gw
                                    # (single PSUM reader in phase B; ACT keeps gelu)
                                    yo = epi_pool.tile([P, C], F32, tag="yo")
                                    nc.vector.scalar_tensor_tensor(
                                        yo[:], py[:], b2_sb[:, mo:mo + 1], gw_sb[:],
                                        op0=mybir.AluOpType.add,
                                        op1=mybir.AluOpType.mult)
                                    # store via SWDGE (gpsimd) so the compute-gated
                                    # store's sem-wait never blocks the HWDGE load
                                    # rings at rep/expert boundaries
                                    nc.gpsimd.dma_start(ygs[j].ap()[mo], yo[:])
            if tick is not None:
                # tiny I/O so the bench program has an ExternalOutput
                nc.sync.dma_start(tick.ap()[:], tk[0:1, 0:2])
    nc.compile()
    return nc


def _assign_experts(counts):
    """LPT + swap refinement: 3 experts per core, minimizing the max core
    load (sum of even-rounded token counts).  Beats the shared-caps SPMD
    grouping because each core's program is sized to its own experts."""
    eff = [max(2, int(c + (c & 1))) for c in counts]
    order = np.argsort(-np.asarray(eff), kind="stable")
    loads = [0] * N_CORES
    assign = [[] for _ in range(N_CORES)]
    for e in order:
        cand = min((i for i in range(N_CORES) if len(assign[i]) < SLOTS),
                   key=lambda i: loads[i])
        assign[cand].append(int(e))
        loads[cand] += eff[e]
    for _ in range(64):  # pairwise-swap refinement of the max core
        mi = int(np.argmax(loads))
        best = None
        for oj in range(N_CORES):
            if oj == mi:
                continue
            for a in range(SLOTS):
                for b in range(SLOTS):
                    d = eff[assign[mi][a]] - eff[assign[oj][b]]
                    if d <= 0:
                        continue
                    new_mx = max(loads[mi] - d, loads[oj] + d)
                    if new_mx < loads[mi] and (best is None or new_mx < best[0]):
                        best = (new_mx, oj, a, b, d)
        if best is None:
            break
        _, oj, a, b, d = best
        assign[mi][a], assign[oj][b] = assign[oj][b], assign[mi][a]
        loads[mi] -= d
        loads[oj] += d
    for c in range(N_CORES):  # biggest expert in slot 0 (matches preload order)
        assign[c].sort(key=lambda e: -eff[e])
    return assign, loads


_jit_cache: dict = {}


def _core_runner(key, nc, core_idx):
    """Single-core reusable PJRT runner for one compiled Bass program,
    pinned to jax device core_idx (mirrors bass2jax.run_bass_via_pjrt)."""
    ck = (key, core_idx)
    if ck in _jit_cache:
        return _jit_cache[ck]
    import jax
    from jax.sharding import Mesh, PartitionSpec
    from jax.experimental.shard_map import shard_map
    from concourse.bass2jax import (
        _bass_exec_p, install_neuronx_cc_hook, partition_id_tensor,
    )
    install_neuronx_cc_hook()

    partition_name = (nc.partition_id_tensor.name
                      if nc.partition_id_tensor else None)
    in_names, out_names, out_avals, zero_specs = [], [], [], []
    for alloc in nc.m.functions[0].allocations:
        if not isinstance(alloc, mybir.MemoryLocationSet):
            continue
        name = alloc.memorylocations[0].name
        if alloc.kind == "ExternalInput":
            if name != partition_name:
                in_names.append(name)
        elif alloc.kind == "ExternalOutput":
            shape = tuple(alloc.tensor_shape)
            dtype = mybir.dt.np(alloc.dtype)
            out_names.append(name)
            out_avals.append(jax.core.ShapedArray(shape, dtype))
            zero_specs.append((shape, dtype))
    n_params = len(in_names)
    n_outs = len(out_avals)
    all_in_names = in_names + out_names + (
        [partition_name] if partition_name else [])
    donate = tuple(range(n_params, n_params + n_outs))

    def _body(*args):
        operands = list(args)
        if partition_name is not None:
            operands.append(partition_id_tensor())
        return tuple(_bass_exec_p.bind(
            *operands,
            out_avals=tuple(out_avals),
            in_names=tuple(all_in_names),
            out_names=tuple(out_names),
            lowering_input_output_aliases=(),
            sim_require_finite=True,
            sim_require_nnan=True,
            nc=nc,
        ))

    mesh = Mesh(np.asarray([jax.devices()[core_idx]]), ("core",))
    in_specs = (PartitionSpec("core"),) * (n_params + n_outs)
    out_specs = (PartitionSpec("core"),) * n_outs
    sharded = jax.jit(
        shard_map(_body, mesh=mesh, in_specs=in_specs, out_specs=out_specs,
                  check_rep=False),
        donate_argnums=donate,
        keep_unused=True,
    )

    def run(in_map):
        ins = [np.asarray(in_map[n]) for n in in_names]
        zeros = [np.zeros(shp, dt) for shp, dt in zero_specs]
        outs = sharded(*ins, *zeros)
        return {name: np.asarray(outs[i]) for i, name in enumerate(out_names)}

    _jit_cache[ck] = run
    return run


def _route(x2d, gate_w, gate_b):
    """fp32 gate scores -> top-2 indices -> softmax combine weights."""
    scores = x2d @ gate_w + gate_b                               # [T, E]
    topi = np.argsort(-scores, axis=1, kind="stable")[:, :TOPK]  # [T, 2]
    topv = np.take_along_axis(scores, topi, axis=1)
    g = np.exp(topv - topv.max(axis=1, keepdims=True))
    g = g / g.sum(axis=1, keepdims=True)
    return topi, g.astype(np.float32)


def kernel(x, gate_w, gate_b, w1, b1, w2, b2):
    x = np.ascontiguousarray(np.asarray(x, dtype=np.float32))
    gate_w = np.asarray(gate_w, dtype=np.float32)
    gate_b = np.asarray(gate_b, dtype=np.float32)
    w1 = np.asarray(w1, dtype=np.float32)
    b1 = np.asarray(b1, dtype=np.float32)
    w2 = np.asarray(w2, dtype=np.float32)
    b2 = np.asarray(b2, dtype=np.float32)

    x2d = x.reshape(T, D)
    topi, gates = _route(x2d, gate_w, gate_b)

    # Token list and combine weight per expert (token order preserved).
    idx_e = [np.nonzero(topi == e)[0] for e in range(E)]
    gv_e = []
    for e in range(E):
        rows = topi == e                       # [T, 2] bool, <=1 True per row
        sel = rows.any(axis=1)
        gv_e.append(gates[sel, :][rows[sel, :]].astype(np.float32))
    counts = np.array([len(i) for i in idx_e])

    # Balance experts over cores: 3 experts per core, per-core program sized
    # to its own experts' counts (LPT + swap refinement minimizes the max
    # core load, ~3.5% fewer padded columns than shared SPMD capacities).
    assign, loads = _assign_experts(counts)
    core_caps = [tuple(max(2, int(counts[e] + (counts[e] & 1)))
                       for e in assign[c]) for c in range(N_CORES)]

    # global e3m4 weight scales (identical across experts: iid uniform)
    s1 = float(np.abs(w1).max() / E3_MAX)
    s2 = float(np.abs(w2).max() / E3_MAX)
    s1k = np.float32(s1).item()

    # bench hooks for test.py: worst core's caps dominate the ensemble
    worst = int(np.argmax(loads))
    globals()["_bench_caps"] = core_caps[worst]
    globals()["_bench_s1"] = s1

    for caps_c in dict.fromkeys(core_caps):  # build distinct programs
        key = (caps_c, s1k)
        if key not in _program_cache:
            _program_cache[key] = _build_program(caps_c, s1=s1)

    xTb = np.ascontiguousarray(x2d.T).astype(NP_BF16)      # [D, T] bf16
    # e3m4-quantized weights (values in +-15.5, dequant scales fold into the
    # ACT scale (mm1) and the host-side gw/b2 rescale (mm2))
    w1q = np.clip(w1 / s1, -E3_MAX, E3_MAX).astype(NP_F8E3)
    w2q = np.clip(w2 / s2, -E3_MAX, E3_MAX).astype(NP_F8E3)
    in_maps = []
    for c in range(N_CORES):
        m = {}
        w1c = np.empty((SLOTS, G1, P, W1G, KT1, P), NP_F8E3)
        w2c = np.empty((SLOTS, G2, P, W2G, KT2, P), NP_F8E3)
        b1c = np.empty((SLOTS, P, MT1), np.float32)
        b2c = np.empty((SLOTS, P, MT2), np.float32)
        for j in range(SLOTS):
            e = int(assign[c][j])
            C = core_caps[c][j]
            n = int(counts[e])
            xg = np.zeros((P, KT1, C), NP_BF16)
            xg[:, :, :n] = xTb[:, idx_e[e]].reshape(KT1, P, n).transpose(1, 0, 2)
            m[f"xg{j}"] = xg
            gw = np.zeros((C,), np.float32)
            gw[:n] = gv_e[e] * s2
            m[f"gw{j}"] = np.broadcast_to(gw, (P, C)).copy()
            # weight tiles in the exact SBUF layouts for single clean DMAs
            t1 = w1q[e].reshape(KT1, P, MT1, P).transpose(2, 1, 0, 3)
            w1c[j] = t1.reshape(G1, W1G, P, KT1, P).transpose(0, 2, 1, 3, 4)
            t2 = w2q[e].reshape(KT2, P, MT2, P).transpose(2, 1, 0, 3)
            w2c[j] = t2.reshape(G2, W2G, P, KT2, P).transpose(0, 2, 1, 3, 4)
            b1c[j] = b1[e].reshape(MT1, P).T
            b2c[j] = (b2[e] / s2).reshape(MT2, P).T
        m["w1t"] = w1c
        m["w2t"] = w2c
        m["b1t"] = b1c
        m["b2t"] = b2c
        in_maps.append(m)

    # run all 8 per-core programs concurrently (jit compile + dispatch per
    # device; threads let the first-call NEFF compiles overlap too)
    from concurrent.futures import ThreadPoolExecutor

    def _run_core(c):
        run = _core_runner((core_caps[c], s1k), _program_cache[(core_caps[c], s1k)], c)
        return run(in_maps[c])

    with ThreadPoolExecutor(N_CORES) as ex:
        results = list(ex.map(_run_core, range(N_CORES)))

    # Combine: scatter-add each expert's weighted outputs back to tokens.
    out = np.zeros((T, D), np.float32)
    for c in range(N_CORES):
        for j in range(SLOTS):
            e = int(assign[c][j])
            n = int(counts[e])
            yg = results[c][f"yg{j}"].reshape(D, core_caps[c][j])
            out[idx_e[e], :] += yg[:, :n].T
    return out.reshape(B, S, D)



# revision 14
# speedup vs baseline: 1.1796x; 1.0681x over previous
"""MoE layer (24 experts, top-2 routing) on 8 Trainium2 NeuronCores.

Expert-parallel sharding: the host computes the gate routing (scores -> top-2
-> softmax combine weights), then dispatches each expert's tokens to the core
that owns the expert (3 experts per core).  Each core runs its own Bass/Tile
program that, for each of its 3 expert slots, computes

    H^T[f, t] = gelu(w1^T-contract(x^T) + b1)      (MM1, K = d_model = 1024)
    Y^T[d, t] = w2^T-contract(H^T) + b2            (MM2, K = d_ff    = 4096)
    out       = Y^T * gate_weight[t]

with tokens on the matmul FREE dim, so per-expert token counts need no
128-padding.  The host scatter-adds the per-expert outputs back into the
[T, d] output (the "combine" side of the all-to-all).

Dtypes: weights are fp8 E3M4 (4 mantissa bits; bass allows mixed-dtype
matmuls, so the moving operands x and h stay bf16 and the PE still streams 1
column/cycle).  This (a) halves the dominant HBM traffic vs bf16 weights
(50 -> 25 MB/core/pass), and (b) cuts PE multiplier toggling - together they
eliminate the sustained power throttle (~283 us sustained with bf16 weights
vs ~235 us burst; with e3m4 weights sustained == burst).  The e3m4 dequant
scales are global (experts iid uniform) and fold into existing epilogues at
zero device cost: MM1's into the ACT gelu `scale` immediate, MM2's into the
host-side gate-weight/bias rescale.  End-to-end rel-err 1.75e-2 (gate 2e-2;
e4m3 weights measure 3.5e-2, all-e4m3 5.2e-2, both over).  Biases, gate
weights, PSUM, and the output stay fp32.

Per-core programs (not SPMD): experts are packed 3-per-core by an LPT +
pairwise-swap assignment minimizing the max core load (sum of even-rounded
counts), and each core's program is compiled for exactly its own three
capacities - the ensemble runs at the worst core's load (~1028 columns vs
1072 for shared-capacity SPMD, a 4% saving; the 3-way partition bound is
1026).  The 8 programs compile in ~20 s (threaded NEFF compiles overlap).

Weight DMAs are batched 1 MiB transfers (8 MM1 f-tiles / 2 MM2 d-tiles per
DMA), alternating between the two HWDGE rings (SP and ACT issuing engines);
output stores go through SWDGE (gpsimd) so their compute-gated semaphore
waits never block the load rings.  The MM2 epilogue is a single fused DVE op
reading PSUM: yo = (py + b2') * (gate * s2).

Per pass the worst core streams 512*1028 ~ 526k PE columns (~219 us at
2.4 GHz) + ~7.7 us of NX dispatch for 1536 LDWEIGHTS+MATMUL pairs; measured
~229 us/pass sustained (within ~1% of that floor).

Host-side work is routing/dispatch/combine only (index math, gather,
scatter-add); all FLOPs of the MoE layer itself (both matmuls, gelu, biases,
gate weighting) run on device.
"""

import os
import sys

for _p in ("/opt/trn_rl_repo", "/root/.axon_site/_ro/trn_rl_repo"):
    if _p not in sys.path:
        sys.path.append(_p)

import ml_dtypes
import numpy as np

import concourse.tile as tile
from concourse import bacc, mybir
from concourse.bass_utils import run_bass_kernel_spmd

B, S, D, FF, E, TOPK = 4, 1024, 1024, 4096, 24, 2
T = B * S
P = 128
KT1 = D // P     # 8  k-subtiles for MM1
MT1 = FF // P    # 32 f-tiles (MM1 output partition tiles)
KT2 = FF // P    # 32 k-subtiles for MM2
MT2 = D // P     # 8  d-tiles (MM2 output partition tiles)
W1G = 8          # MM1 f-tiles per weight DMA (2 MiB bf16 per transfer)
G1 = MT1 // W1G  # 4 w1 DMA groups
W2G = 2          # MM2 d-tiles per weight DMA (2 MiB bf16 per transfer)
G2 = MT2 // W2G  # 4 w2 DMA groups
N_CORES = 8
SLOTS = E // N_CORES  # 3 experts per core

BF16 = mybir.dt.bfloat16
F8E3 = mybir.dt.float8e3
F32 = mybir.dt.float32
NP_BF16 = ml_dtypes.bfloat16
NP_F8E3 = ml_dtypes.float8_e3m4

# Weights are stored and matmul'd as fp8 E3M4 (4 mantissa bits) against bf16
# moving operands - the PE allows mixed dtypes and runs at bf16 speed (1
# col/cycle), so this halves the dominant weight HBM traffic (50->25 MB/core
# per pass) and cuts PE multiplier toggling, while the global dequant scales
# fold into existing epilogues (ACT `scale` for MM1, gate-weight multiply for
# MM2) at zero device cost.  Measured end-to-end rel-err ~1.75e-2 (gate 2e-2);
# weights uniform +-1/sqrt(D) quantize to +-15.5 with a single global scale.
E3_MAX = 15.5

_program_cache: dict = {}


def _build_program(caps, s1=1.0, loop_reps=None, bench_internal=False, hw_loop=0):
    """One SPMD program: SLOTS expert slots with token capacities caps[j].

    loop_reps: replicate the body N times (benchmark-only, to measure the
    steady-state device time via a wall-clock slope over N).
    bench_internal: benchmark-only - every tensor lives in internal DRAM
    scratch (plus one tiny ExternalOutput so the program has I/O), so
    wall-clock timing excludes host<->device shipping while keeping
    an identical per-rep instruction stream and DMA traffic.
    hw_loop: benchmark-only - wrap the loop_reps unrolled reps in a hardware
    For_i loop of this many iterations (total reps = loop_reps * hw_loop).
    The relay's ~100 ms per-call overhead partially overlaps device
    execution, so honest wall-clock slopes need hundreds of ms of device
    time per call; the HW loop's back-edge barrier adds a small real cost
    per iteration (measured time is, if anything, pessimistic).
    """
    nc = bacc.Bacc("TRN2", target_bir_lowering=False, debug=False)

    kin = "Internal" if bench_internal else "ExternalInput"
    kout = "Internal" if bench_internal else "ExternalOutput"
    sfx = "_int" if bench_internal else ""
    w1t = nc.dram_tensor("w1t" + sfx, (SLOTS, G1, P, W1G, KT1, P), F8E3, kind=kin)
    w2t = nc.dram_tensor("w2t" + sfx, (SLOTS, G2, P, W2G, KT2, P), F8E3, kind=kin)
    b1t = nc.dram_tensor("b1t" + sfx, (SLOTS, P, MT1), F32, kind=kin)
    b2t = nc.dram_tensor("b2t" + sfx, (SLOTS, P, MT2), F32, kind=kin)
    xgs = [nc.dram_tensor(f"xg{j}" + sfx, (P, KT1, caps[j]), BF16, kind=kin)
           for j in range(SLOTS)]
    gws = [nc.dram_tensor(f"gw{j}" + sfx, (P, caps[j]), F32, kind=kin)
           for j in range(SLOTS)]
    ygs = [nc.dram_tensor(f"yg{j}" + sfx, (MT2, P, caps[j]), F32, kind=kout)
           for j in range(SLOTS)]
    tick = (nc.dram_tensor("tick", (1, 2), F32, kind="ExternalOutput")
            if bench_internal else None)

    import contextlib

    with tile.TileContext(nc) as tc:
        with tc.tile_pool(name="xg", bufs=SLOTS + 1) as xg_pool, \
             tc.tile_pool(name="gw", bufs=SLOTS + 1) as gw_pool, \
             tc.tile_pool(name="bias", bufs=SLOTS + 1) as bias_pool, \
             tc.tile_pool(name="w1", bufs=4) as w1_pool, \
             tc.tile_pool(name="w2", bufs=4) as w2_pool, \
             tc.tile_pool(name="h", bufs=MT1 + 2) as h_pool, \
             tc.tile_pool(name="epi", bufs=4) as epi_pool, \
             tc.tile_pool(name="psa", bufs=4, space="PSUM") as psa, \
             tc.tile_pool(name="psb", bufs=4, space="PSUM") as psb:
            if tick is not None:
                tk = gw_pool.tile([P, 2], F32, tag="tick")
                nc.vector.memset(tk[:], 0.0)
            loop_cm = (tc.For_i(0, hw_loop, 1,
                               hint_engines=(mybir.EngineType.PE,))
                       if hw_loop else contextlib.nullcontext())
            with loop_cm:
                for _rep in range(loop_reps or 1):
                        dma_rr = [0]

                        def wdma(dst, src):
                            # alternate weight DMAs across the two HWDGE rings
                            eng = nc.scalar if (dma_rr[0] % 2) else nc.sync
                            dma_rr[0] += 1
                            eng.dma_start(dst, src)

                        # Slot preloads: activations/gates ride the SCALAR
                        # ring so the first w1 DMA leads the SYNC ring and the
                        # PE's first matmul waits only max(xg0, w1g0), not
                        # their sum.  Slots 1-2 preload lazily during the
                        # previous expert's compute (plenty of slack).
                        slot_in = []

                        def preload(j):
                            C = caps[j]
                            xg_sb = xg_pool.tile([P, KT1, C], BF16, tag="xg")
                            nc.scalar.dma_start(xg_sb[:], xgs[j].ap()[:])
                            gw_sb = gw_pool.tile([P, C], F32, tag="gw")
                            nc.scalar.dma_start(gw_sb[:], gws[j].ap()[:])
                            b1_sb = bias_pool.tile([P, MT1], F32, tag="b1")
                            nc.sync.dma_start(b1_sb[:], b1t.ap()[j])
                            b2_sb = bias_pool.tile([P, MT2], F32, tag="b2")
                            nc.sync.dma_start(b2_sb[:], b2t.ap()[j])
                            slot_in.append((xg_sb, gw_sb, b1_sb, b2_sb))

                        preload(0)
                        for j in range(SLOTS):
                            C = caps[j]
                            xg_sb, gw_sb, b1_sb, b2_sb = slot_in[j]

                            # Phase A: H^T tiles, one 128-row f-tile at a time.
                            h_tiles = []
                            for g in range(G1):
                                w1_sb = w1_pool.tile([P, W1G, KT1, P], F8E3, tag="w1")
                                wdma(w1_sb[:], w1t.ap()[j, g])
                                for mi in range(W1G):
                                    m = g * W1G + mi
                                    ph = psa.tile([P, C], F32, tag="psa")
                                    for k in range(KT1):
                                        nc.tensor.matmul(ph[:], w1_sb[:, mi, k, :],
                                                         xg_sb[:, k, :],
                                                         start=(k == 0),
                                                         stop=(k == KT1 - 1))
                                    h_sb = h_pool.tile([P, C], BF16, tag="h")
                                    nc.scalar.activation(h_sb[:], ph[:],
                                                         mybir.ActivationFunctionType.Gelu,
                                                         bias=b1_sb[:, m:m + 1],
                                                         scale=float(s1))
                                    h_tiles.append(h_sb)

                            # Phase B: Y^T tiles; epilogue adds b2, scales by gate.
                            for go in range(G2):
                                w2_sb = w2_pool.tile([P, W2G, KT2, P], F8E3, tag="w2")
                                wdma(w2_sb[:], w2t.ap()[j, go])
                                if go == 0 and j + 1 < SLOTS:
                                    preload(j + 1)
                                for mi in range(W2G):
                                    mo = go * W2G + mi
                                    py = psb.tile([P, C], F32, tag="psb")
                                    for k in range(KT2):
                                        nc.tensor.matmul(py[:], w2_sb[:, mi, k, :],
                                                         h_tiles[k][:],
                                                         start=(k == 0),
                                                         stop=(k == KT2 - 1))
                                    # fused epilogue on DVE: yo = (py + b2) * gw
                                    # (single PSUM reader in phase B; ACT keeps gelu)
                                    yo = epi_pool.tile([P, C], F32, tag="yo")
                                    nc.vector.scalar_tensor_tensor(
                                        yo[:], py[:], b2_sb[:, mo:mo + 1], gw_sb[:],
                                        op0=mybir.AluOpType.add,
                                        op1=mybir.AluOpType.mult)
                                    # store via SWDGE (gpsimd) so the compute-gated
                                    # store's sem-wait never blocks the HWDGE load
                                    # rings at rep/expert boundaries
                                    nc.gpsimd.dma_start(ygs[j].ap()[mo], yo[:])
            if tick is not None:
                # tiny I/O so the bench program has an ExternalOutput
                nc.sync.dma_start(tick.ap()[:], tk[0:1, 0:2])
    nc.compile()
    return nc


def _assign_experts(counts):
    """LPT + swap refinement: 3 experts per core, minimizing the max core
    load (sum of even-rounded token counts).  Beats the shared-caps SPMD
    grouping because each core's program is sized to its own experts."""
    eff = [max(2, int(c + (c & 1))) for c in counts]
    order = np.argsort(-np.asarray(eff), kind="stable")
    loads = [0] * N_CORES
    assign = [[] for _ in range(N_CORES)]
    for e in order:
        cand = min((i for i in range(N_CORES) if len(assign[i]) < SLOTS),
                   key=lambda i: loads[i])
        assign[cand].append(int(e))
        loads[cand] += eff[e]
    for _ in range(64):  # pairwise-swap refinement of the max core
        mi = int(np.argmax(loads))
        best = None
        for oj in range(N_CORES):
            if oj == mi:
                continue
            for a in range(SLOTS):
                for b in range(SLOTS):
                    d = eff[assign[mi][a]] - eff[assign[oj][b]]
                    if d <= 0:
                        continue
                    new_mx = max(loads[mi] - d, loads[oj] + d)
                    if new_mx < loads[mi] and (best is None or new_mx < best[0]):
                        best = (new_mx, oj, a, b, d)
        if best is None:
            break
        _, oj, a, b, d = best
        assign[mi][a], assign[oj][b] = assign[oj][b], assign[mi][a]
        loads[mi] -= d
        loads[oj] += d
    for c in range(N_CORES):  # biggest expert in slot 0 (matches preload order)
        assign[c].sort(key=lambda e: -eff[e])
    return assign, loads


_jit_cache: dict = {}


def _core_runner(key, nc, core_idx):
    """Single-core reusable PJRT runner for one compiled Bass program,
    pinned to jax device core_idx (mirrors bass2jax.run_bass_via_pjrt)."""
    ck = (key, core_idx)
    if ck in _jit_cache:
        return _jit_cache[ck]
    import jax
    from jax.sharding import Mesh, PartitionSpec
    from jax.experimental.shard_map import shard_map
    from concourse.bass2jax import (
        _bass_exec_p, install_neuronx_cc_hook, partition_id_tensor,
    )
    install_neuronx_cc_hook()

    partition_name = (nc.partition_id_tensor.name
                      if nc.partition_id_tensor else None)
    in_names, out_names, out_avals, zero_specs = [], [], [], []
    for alloc in nc.m.functions[0].allocations:
        if not isinstance(alloc, mybir.MemoryLocationSet):
            continue
        name = alloc.memorylocations[0].name
        if alloc.kind == "ExternalInput":
            if name != partition_name:
                in_names.append(name)
        elif alloc.kind == "ExternalOutput":
            shape = tuple(alloc.tensor_shape)
            dtype = mybir.dt.np(alloc.dtype)
            out_names.append(name)
            out_avals.append(jax.core.ShapedArray(shape, dtype))
            zero_specs.append((shape, dtype))
    n_params = len(in_names)
    n_outs = len(out_avals)
    all_in_names = in_names + out_names + (
        [partition_name] if partition_name else [])
    donate = tuple(range(n_params, n_params + n_outs))

    def _body(*args):
        operands = list(args)
        if partition_name is not None:
            operands.append(partition_id_tensor())
        return tuple(_bass_exec_p.bind(
            *operands,
            out_avals=tuple(out_avals),
            in_names=tuple(all_in_names),
            out_names=tuple(out_names),
            lowering_input_output_aliases=(),
            sim_require_finite=True,
            sim_require_nnan=True,
            nc=nc,
        ))

    mesh = Mesh(np.asarray([jax.devices()[core_idx]]), ("core",))
    in_specs = (PartitionSpec("core"),) * (n_params + n_outs)
    out_specs = (PartitionSpec("core"),) * n_outs
    sharded = jax.jit(
        shard_map(_body, mesh=mesh, in_specs=in_specs, out_specs=out_specs,
                  check_rep=False),
        donate_argnums=donate,
        keep_unused=True,
    )

    def run(in_map):
        ins = [np.asarray(in_map[n]) for n in in_names]
        zeros = [np.zeros(shp, dt) for shp, dt in zero_specs]
        outs = sharded(*ins, *zeros)
        return {name: np.asarray(outs[i]) for i, name in enumerate(out_names)}

    _jit_cache[ck] = run
    return run


def _route(x2d, gate_w, gate_b):
    """fp32 gate scores -> top-2 indices -> softmax combine weights."""
    scores = x2d @ gate_w + gate_b                               # [T, E]
    topi = np.argsort(-scores, axis=1, kind="stable")[:, :TOPK]  # [T, 2]
    topv = np.take_along_axis(scores, topi, axis=1)
    g = np.exp(topv - topv.max(axis=1, keepdims=True))
    g = g / g.sum(axis=1, keepdims=True)
    return topi, g.astype(np.float32)


def kernel(x, gate_w, gate_b, w1, b1, w2, b2):
    x = np.ascontiguousarray(np.asarray(x, dtype=np.float32))
    gate_w = np.asarray(gate_w, dtype=np.float32)
    gate_b = np.asarray(gate_b, dtype=np.float32)
    w1 = np.asarray(w1, dtype=np.float32)
    b1 = np.asarray(b1, dtype=np.float32)
    w2 = np.asarray(w2, dtype=np.float32)
    b2 = np.asarray(b2, dtype=np.float32)

    x2d = x.reshape(T, D)
    topi, gates = _route(x2d, gate_w, gate_b)

    # Token list and combine weight per expert (token order preserved).
    idx_e = [np.nonzero(topi == e)[0] for e in range(E)]
    gv_e = []
    for e in range(E):
        rows = topi == e                       # [T, 2] bool, <=1 True per row
        sel = rows.any(axis=1)
        gv_e.append(gates[sel, :][rows[sel, :]].astype(np.float32))
    counts = np.array([len(i) for i in idx_e])

    # Balance experts over cores: 3 experts per core, per-core program sized
    # to its own experts' counts (LPT + swap refinement minimizes the max
    # core load, ~3.5% fewer padded columns than shared SPMD capacities).
    assign, loads = _assign_experts(counts)
    core_caps = [tuple(max(2, int(counts[e] + (counts[e] & 1)))
                       for e in assign[c]) for c in range(N_CORES)]

    # global e3m4 weight scales (identical across experts: iid uniform)
    s1 = float(np.abs(w1).max() / E3_MAX)
    s2 = float(np.abs(w2).max() / E3_MAX)
    s1k = np.float32(s1).item()

    # bench hooks for test.py: worst core's caps dominate the ensemble
    worst = int(np.argmax(loads))
    globals()["_bench_caps"] = core_caps[worst]
    globals()["_bench_s1"] = s1

    for caps_c in dict.fromkeys(core_caps):  # build distinct programs
        key = (caps_c, s1k)
        if key not in _program_cache:
            _program_cache[key] = _build_program(caps_c, s1=s1)

    xTb = np.ascontiguousarray(x2d.T).astype(NP_BF16)      # [D, T] bf16
    # e3m4-quantized weights (values in +-15.5, dequant scales fold into the
    # ACT scale (mm1) and the host-side gw/b2 rescale (mm2))
    w1q = np.clip(w1 / s1, -E3_MAX, E3_MAX).astype(NP_F8E3)
    w2q = np.clip(w2 / s2, -E3_MAX, E3_MAX).astype(NP_F8E3)
    in_maps = []
    for c in range(N_CORES):
        m = {}
        w1c = np.empty((SLOTS, G1, P, W1G, KT1, P), NP_F8E3)
        w2c = np.empty((SLOTS, G2, P, W2G, KT2, P), NP_F8E3)
        b1c = np.empty((SLOTS, P, MT1), np.float32)
        b2c = np.empty((SLOTS, P, MT2), np.float32)
        for j in range(SLOTS):
            e = int(assign[c][j])
            C = core_caps[c][j]
            n = int(counts[e])
            xg = np.zeros((P, KT1, C), NP_BF16)
            xg[:, :, :n] = xTb[:, idx_e[e]].reshape(KT1, P, n).transpose(1, 0, 2)
            m[f"xg{j}"] = xg
            gw = np.zeros((C,), np.float32)
            gw[:n] = gv_e[e] * s2
            m[f"gw{j}"] = np.broadcast_to(gw, (P, C)).copy()
            # weight tiles in the exact SBUF layouts for single clean DMAs
            t1 = w1q[e].reshape(KT1, P, MT1, P).transpose(2, 1, 0, 3)
            w1c[j] = t1.reshape(G1, W1G, P, KT1, P).transpose(0, 2, 1, 3, 4)
            t2 = w2q[e].reshape(KT2, P, MT2, P).transpose(2, 1, 0, 3)
            w2c[j] = t2.reshape(G2, W2G, P, KT2, P).transpose(0, 2, 1, 3, 4)
            b1c[j] = b1[e].reshape(MT1, P).T
            b2c[j] = (b2[e] / s2).reshape(MT2, P).T
        m["w1t"] = w1c
        m["w2t"] = w2c
        m["b1t"] = b1c
        m["b2t"] = b2c
        in_maps.append(m)

    # run all 8 per-core programs concurrently (jit compile + dispatch per
    # device; threads let the first-call NEFF compiles overlap too)
    from concurrent.futures import ThreadPoolExecutor

    def _run_core(c):
        run = _core_runner((core_caps[c], s1k), _program_cache[(core_caps[c], s1k)], c)
        return run(in_maps[c])

    with ThreadPoolExecutor(N_CORES) as ex:
        results = list(ex.map(_run_core, range(N_CORES)))

    # Combine: scatter-add each expert's weighted outputs back to tokens.
    out = np.zeros((T, D), np.float32)
    for c in range(N_CORES):
        for j in range(SLOTS):
            e = int(assign[c][j])
            n = int(counts[e])
            yg = results[c][f"yg{j}"].reshape(D, core_caps[c][j])
            out[idx_e[e], :] += yg[:, :n].T
    return out.reshape(B, S, D)

